# revision 14
# baseline (speedup 1.0000x reference)
"""Trainium2 Bass kernel for the BasicRNN problem — v3: 2-way batch x 4-way
W-column sharding with two-sub-batch software pipelining.

Each group of 4 cores handles one batch half (128 cols); core g' owns 8 of
the 32 state row-chunks (2 S + 4 I + 2 O). The 128 batch cols are split into
two sub-batches of 64. Per step the core computes next-state for h0 (256
matmuls of 64-wide, ~6.8us), launches the h0 AllGather (128KB-in, ~7.4us),
computes h1 while h0's gather flies, launches h1's gather, and so on — the
PE and the collective queue stay concurrently busy instead of serializing.

Explicit dep edges chain each sub-step's first matmul to the previous
sub-step's last matmul so the scheduler cannot interleave a blocked
sub-step's matmuls (an AG wait) ahead of ready work.

A warm-up AllGather sized exactly like the step gathers (128KB-in) runs at
t=0: the first collective cannot execute before ~60us of runtime init no
matter what, so it also pre-stages the real transfer plan.

E = in_w @ x_sh.T is computed fully per core (replicated in the group) so
step 1 needs no communication; injection at s=5 adds the core's own 2 S
chunks of E. Classifier: core g' computes out.T rows [250g', 250g'+250) per
sub-batch; host assembles.
"""

import numpy as np

S_DIM, I_DIM, O_DIM = 1024, 2048, 1024
TOTAL = 4096
INPUT_DIM, NUM_CLASSES, BATCH = 2048, 1000, 256
NC = 8
G = 4                      # group size (batch half handled by 4 cores)
BSH = BATCH // 2           # 128 batch columns per group
SB = BSH // 2              # 64-column sub-batch
KC = TOTAL // 128          # 32 k-chunks of 128
MB = 8                     # m-blocks per core
CLS_M = NUM_CLASSES // G   # 250 classifier rows per core

def _block_a(g):
    return [2 * g, 2 * g + 1, 8 + 4 * g, 9 + 4 * g]

def _block_b(g):
    return [10 + 4 * g, 11 + 4 * g, 24 + 2 * g, 25 + 2 * g]

# one AG per sub-batch gathers each core's full 8-block slab, so device
# k-chunk order is rank-major over [block_a(g) + block_b(g)]
BLOCK_ORDER = sum((_block_a(g) + _block_b(g) for g in range(G)), [])

FILL_WARM = 500   # fillers after E+step1 (cover pre-collective init window)
FILL_S = 130      # fillers after each sub-step's gather launch

_cache = {}


def _split_excess_waits(nc, mybir, limit=1, nop_limit=1):
    """This walrus build rejects multiple sync-waits on most instruction
    structs and any wait on Drain/ISA. Move excess waits onto preceding
    same-engine nops."""
    counter = [0]

    def make_nop(engine, waits):
        counter[0] += 1
        nop = mybir.InstNoOp(name=f"I-ws{counter[0]}", engine=engine)
        nop.sync_info = mybir.SyncInfo(on_wait=list(waits), on_update=[])
        return nop

    for fn in nc.m.functions:
        for bb in fn.blocks:
            out = []
            changed = False
            for inst in bb.instructions:
                si = getattr(inst, "sync_info", None)
                waits = list(si.on_wait) if si is not None and si.on_wait else []
                lim = 0 if isinstance(inst, (mybir.InstDrain, mybir.InstISA)) else limit
                if len(waits) > lim:
                    keep = waits[-lim:] if lim else []
                    excess = waits[: len(waits) - lim]
                    for g in range(0, len(excess), nop_limit):
                        nop = make_nop(inst.engine, excess[g : g + nop_limit])
                        nc.register_instruction(nop, overwrite=True)
                        out.append(nop)
                    si.on_wait = keep
                    changed = True
                out.append(inst)
            if changed:
                bb.instructions = out


def _build(T):
    import concourse.bass as bass
    import concourse.tile as tile
    from concourse import mybir
    from concourse.tile_rust import add_dep_helper

    f32 = mybir.dt.float32
    bf16 = mybir.dt.bfloat16
    B = BSH

    nc = bass.Bass()
    wc = nc.dram_tensor("wc", [TOTAL, 128 * MB], bf16, kind="ExternalInput")
    xTc = nc.dram_tensor("xTc", [INPUT_DIM, B], bf16, kind="ExternalInput")
    inwT = nc.dram_tensor("inwT", [INPUT_DIM, S_DIM], bf16, kind="ExternalInput")
    inb = nc.dram_tensor("inb", [128, 8], f32, kind="ExternalInput")
    inwTo = nc.dram_tensor("inwTo", [INPUT_DIM, 256], bf16, kind="ExternalInput")
    inbo = nc.dram_tensor("inbo", [128, 2], f32, kind="ExternalInput")
    outwTc = nc.dram_tensor("outwTc", [256, NUM_CLASSES], bf16, kind="ExternalInput")
    ident = nc.dram_tensor("ident", [128, 128], bf16, kind="ExternalInput")
    warm = nc.dram_tensor("warm", [128, 32], bf16, kind="ExternalInput")
    out_t = nc.dram_tensor("out_t", [NUM_CLASSES, B], f32, kind="ExternalOutput")

    RG = [[0, 1, 2, 3], [4, 5, 6, 7]]

    def ag(ins_ap, out_ap):
        nc.gpsimd.collective_compute(
            "AllGather", mybir.AluOpType.bypass,
            replica_groups=RG, ins=[ins_ap], outs=[out_ap],
        )

    with tile.TileContext(nc) as tc:
        with (
            tc.tile_pool(name="wp", bufs=1) as wp,
            tc.tile_pool(name="pers", bufs=1) as pers,
            tc.tile_pool(name="state", bufs=2) as stp,
            tc.tile_pool(name="res", bufs=2) as resp,
            tc.tile_pool(name="psum", bufs=1, space="PSUM") as psp,
            tc.tile_pool(name="psum1", bufs=1, space="PSUM") as psp1,
            tc.tile_pool(name="dram", bufs=2, space="DRAM") as dram,
        ):
            # warm-up collective, sized identically to the step gathers
            warm_in = dram.tile([1024, SB], bf16, name="warmin", tag="warmin")
            nc.scalar.dma_start(warm_in[:], xTc[0:1024, 0:SB])
            warm_out = dram.tile([4096, SB], bf16, name="warmout", tag="warmout")
            ag(warm_in.opt(), warm_out.opt())

            # --- static weights/consts into SBUF ---
            wt = wp.tile([128, KC * 1024], bf16, name="wt", tag="wt")
            S_FIRST = [d for d in range(KC) if BLOCK_ORDER[d] < 8] + [
                d for d in range(KC) if BLOCK_ORDER[d] >= 8
            ]
            for d in S_FIRST:
                nc.sync.dma_start(
                    wt[:, d * 1024 : (d + 1) * 1024], wc[128 * d : 128 * (d + 1), :]
                )

            def wslice(k, m):
                return wt[:, k * 1024 + 128 * m : k * 1024 + 128 * (m + 1)]

            xt = pers.tile([128, (INPUT_DIM // 128) * B], bf16, name="xt", tag="xt")
            nc.scalar.dma_start(
                xt[:].rearrange("p (k b) -> p k b", b=B),
                xTc[:].rearrange("(k p) b -> p k b", p=128),
            )
            iwt = pers.tile([128, (INPUT_DIM // 128) * S_DIM], bf16, name="iwt", tag="iwt")
            nc.scalar.dma_start(
                iwt[:].rearrange("p (k m) -> p k m", m=S_DIM),
                inwT[:].rearrange("(k p) m -> p k m", p=128),
            )
            iwo = pers.tile([128, (INPUT_DIM // 128) * 256], bf16, name="iwo", tag="iwo")
            nc.scalar.dma_start(
                iwo[:].rearrange("p (k m) -> p k m", m=256),
                inwTo[:].rearrange("(k p) m -> p k m", p=128),
            )
            id_t = pers.tile([128, 128], bf16, name="ident", tag="ident")
            nc.scalar.dma_start(id_t[:], ident[:])
            inb_t = pers.tile([128, 8], f32, name="inb", tag="inb")
            nc.scalar.dma_start(inb_t[:], inb[:])
            inbo_t = pers.tile([128, 2], f32, name="inbo", tag="inbo")
            nc.scalar.dma_start(inbo_t[:], inbo[:])
            owt = pers.tile([128, 2 * NUM_CLASSES], bf16, name="owt", tag="owt")
            nc.scalar.dma_start(
                owt[:].rearrange("p (k m) -> p k m", m=NUM_CLASSES),
                outwTc[:].rearrange("(k p) m -> p k m", p=128),
            )

            # --- E phase: full relu(E) [1024, B] + own-chunk E for injection ---
            e_t = pers.tile([128, 8 * B], bf16, name="et", tag="et")
            for m in range(8):
                ps_e = psp.tile([128, B], f32, name="pse", tag=f"ps{m % 2}")
                for k in range(INPUT_DIM // 128):
                    nc.tensor.matmul(
                        ps_e[:],
                        iwt[:, k * S_DIM + 128 * m : k * S_DIM + 128 * (m + 1)],
                        xt[:, B * k : B * (k + 1)],
                        start=(k == 0), stop=(k == INPUT_DIM // 128 - 1),
                    )
                nc.scalar.activation(
                    e_t[:, B * m : B * (m + 1)], ps_e[:],
                    mybir.ActivationFunctionType.Relu, bias=inb_t[:, m : m + 1],
                )
            einj = pers.tile([128, 2 * B], bf16, name="einj", tag="einj")
            for m in range(2):
                ps_o = psp.tile([128, B], f32, name="pso", tag=f"ps{m}")
                for k in range(INPUT_DIM // 128):
                    nc.tensor.matmul(
                        ps_o[:],
                        iwo[:, k * 256 + 128 * m : k * 256 + 128 * (m + 1)],
                        xt[:, B * k : B * (k + 1)],
                        start=(k == 0), stop=(k == INPUT_DIM // 128 - 1),
                    )
                nc.scalar.activation(
                    einj[:, B * m : B * (m + 1)], ps_o[:],
                    mybir.ActivationFunctionType.Identity, bias=inbo_t[:, m : m + 1],
                )

            ps_d = psp1.tile([128, 64], f32, name="psd", tag="psd")

            def fill(n, rhs=None):
                r = e_t[:, 0:64] if rhs is None else rhs[:, 0:64]
                for _ in range(n):
                    nc.tensor.matmul(ps_d[:], id_t[:], r, start=True, stop=True)

            S_DEV = [d for d in range(KC) if BLOCK_ORDER[d] < 8]
            last = T - 1
            agout = {}   # (h) -> latest gathered state dram tile
            resa = {}
            prev_mm = None

            def substep(s, h):
                """Compute sub-batch h of step s; relu into resa[h]; gather."""
                nonlocal prev_mm
                m_list = [6, 7] if s == last else list(range(MB))
                inject = s % 5 == 0 and s >= 2
                if s == 1:
                    stsl = lambda d: e_t[:, B * BLOCK_ORDER[d] + SB * h :
                                         B * BLOCK_ORDER[d] + SB * h + SB]
                    klist = S_DEV
                else:
                    stH = stp.tile([128, KC * SB], bf16, name=f"st{h}", tag=f"st{h}")
                    nc.sync.dma_start(
                        stH[:, : 16 * SB].rearrange("p (k b) -> p k b", b=SB),
                        agout[h][:2048, :].rearrange("(k p) b -> p k b", p=128),
                    )
                    nc.scalar.dma_start(
                        stH[:, 16 * SB :].rearrange("p (k b) -> p k b", b=SB),
                        agout[h][2048:, :].rearrange("(k p) b -> p k b", p=128),
                    )
                    stsl = lambda k: stH[:, k * SB : (k + 1) * SB]
                    klist = list(range(KC))
                resa[h] = resp.tile([128, MB * SB], bf16, name=f"resa{h}", tag=f"resa{h}")
                for mi, m in enumerate(m_list):
                    psm = psp.tile([128, SB], f32, name=f"ps{m}", tag=f"ps{m % 4}")
                    for i, k in enumerate(klist):
                        mm = nc.tensor.matmul(
                            psm[:], wslice(k, m), stsl(k),
                            start=(i == 0),
                            stop=(i == len(klist) - 1) and not (inject and m < 2),
                        )
                        if i == 0 and prev_mm is not None:
                            add_dep_helper(
                                mm.ins, prev_mm.ins, reason="substep order"
                            )
                    if inject and m < 2:
                        mm = nc.tensor.matmul(
                            psm[:], id_t[:],
                            einj[:, B * m + SB * h : B * m + SB * h + SB],
                            start=False, stop=True,
                        )
                    prev_mm = mm
                    nc.vector.tensor_relu(resa[h][:, SB * m : SB * (m + 1)], psm[:])
                if s == last:
                    return
                new = dram.tile([1024, SB], bf16, name=f"ag{h}", tag=f"ag{h}")
                nc.sync.dma_start(
                    new[:].rearrange("(m p) b -> p m b", p=128),
                    resa[h][:].rearrange("p (m b) -> p m b", b=SB),
                )
                agout[h] = dram.tile(
                    [4096, SB], bf16, name=f"agout{h}", tag=f"agout{h}"
                )
                ag(new.opt(), agout[h].opt())

            # steps: sub-batches pipelined h0, h1, h0, h1, ...
            for s in range(1, T):
                for h in range(2):
                    substep(s, h)
                    if s == 1 and h == 1:
                        fill(FILL_WARM)
                    elif s != last:
                        fill(FILL_S, rhs=resa[h])

            # --- partial classifier: own 2 O chunks, all classes; host sums
            # the 4 per-core partials per batch half (no final gather) ---
            out_sb = pers.tile([125, 16 * SB], f32, name="outsb", tag="outsb")
            for h in range(2):
                for pc in range(8):
                    ps_c = psp.tile([125, SB], f32, name="psc", tag=f"ps{pc % 4}")
                    for kk in range(2):
                        nc.tensor.matmul(
                            ps_c[:],
                            owt[:, kk * NUM_CLASSES + 125 * pc :
                                kk * NUM_CLASSES + 125 * (pc + 1)],
                            resa[h][:, (6 + kk) * SB : (7 + kk) * SB],
                            start=(kk == 0), stop=(kk == 1),
                        )
                    nc.scalar.activation(
                        out_sb[:, (8 * h + pc) * SB : (8 * h + pc + 1) * SB], ps_c[:],
                        mybir.ActivationFunctionType.Identity,
                    )
                nc.sync.dma_start(
                    out_t[:, SB * h : SB * (h + 1)].rearrange(
                        "(pc p) b -> p pc b", p=125
                    ),
                    out_sb[:, 8 * h * SB : (8 * h + 8) * SB].rearrange(
                        "p (pc b) -> p pc b", b=SB
                    ),
                )

    _split_excess_waits(nc, mybir)
    return nc


def kernel(x, W, in_w, in_b, out_w, out_b, time_steps):
    T = int(time_steps)
    x = np.ascontiguousarray(x, dtype=np.float32)
    W = np.ascontiguousarray(W, dtype=np.float32)
    in_w = np.ascontiguousarray(in_w, dtype=np.float32)
    in_b = np.ascontiguousarray(in_b, dtype=np.float32)
    out_w = np.ascontiguousarray(out_w, dtype=np.float32)
    out_b = np.ascontiguousarray(out_b, dtype=np.float32)

    if T < 2:
        return np.broadcast_to(out_b, (BATCH, NUM_CLASSES)).astype(np.float32).copy()

    import ml_dtypes
    from concourse.bass_utils import run_bass_kernel_spmd

    if T not in _cache:
        _cache[T] = _build(T)
    nc = _cache[T]

    bf = ml_dtypes.bfloat16
    Wd = W.reshape(KC, 128, TOTAL)[BLOCK_ORDER].reshape(TOTAL, TOTAL)
    Wcols = Wd.reshape(TOTAL, KC, 128)
    xT = np.ascontiguousarray(x.T)
    inwT_np = in_w.T.astype(bf)
    outwT = out_w.T.astype(bf)
    ident = np.eye(128, dtype=np.float32).astype(bf)
    warm_np = np.zeros((128, 32), dtype=np.float32).astype(bf)
    inb_np = np.ascontiguousarray(in_b.reshape(8, 128).T)

    in_maps = []
    for c in range(NC):
        h, g = divmod(c, G)
        own = _block_a(g) + _block_b(g)
        wcc = np.ascontiguousarray(
            Wcols[:, own].reshape(TOTAL, 128 * MB)
        ).astype(bf)
        in_maps.append({
            "wc": wcc,
            "xTc": np.ascontiguousarray(xT[:, BSH * h : BSH * (h + 1)]).astype(bf),
            "inwT": inwT_np,
            "inb": inb_np,
            "inwTo": np.ascontiguousarray(inwT_np[:, 256 * g : 256 * (g + 1)]),
            "inbo": np.ascontiguousarray(
                in_b[256 * g : 256 * (g + 1)].reshape(2, 128).T
            ),
            "outwTc": np.ascontiguousarray(outwT[256 * g : 256 * (g + 1), :]),
            "ident": ident,
            "warm": warm_np,
        })
    res = run_bass_kernel_spmd(nc, in_maps, list(range(NC)))
    out = np.empty((BATCH, NUM_CLASSES), dtype=np.float32)
    for H in range(2):
        acc = sum(res.results[4 * H + g]["out_t"] for g in range(G))
        out[BSH * H : BSH * (H + 1), :] = acc.T + out_b
    return out
